# revision 1
# baseline (speedup 1.0000x reference)
"""Trainium2 Bass/Tile kernel for the GatedNode2Edge op.

Computes, for emb (B,C,N), th12_* (E,C), th5_* (E,):
    t_k  = th12_k @ emb[b]                      (E,N)
    m_k  = max(t_k[:,i], t_k[:,j]) pairwise     (E,N,N)
    adj  = relu(2*m_1 + th5_1*I)
    gate = sigmoid(relu(2*m_2 + th5_2*I))
    out  = adj * gate                           (B,E,N,N)

Sharding: the 64 (b,e) channels are split 8-per-core across 8 NeuronCores.

Math restructuring (off-diagonal):
    relu(2*max(a,b)) = max(2*relu(a), 2*relu(b))           (relu monotone)
    sigmoid(max(x,y)) = max(sigmoid(x), sigmoid(y))        (sigmoid monotone)
so with row vectors v = 2*relu(t1), g = sigmoid(2*relu(t2)):
    out[i,j] = max(v_i, v_j) * max(g_i, g_j)
which is ONE fused custom-DVE op per [128, N] output tile:
    out = maxx(Src0, C0) * maxx(Src1, C1)
with Src0 = v broadcast across partitions (PE outer-product), C0 = v column
slice (per-partition scalar), likewise Src1/C1 for g. The true diagonal is
patched with copy_predicated against an identity mask. Sigmoid runs once per
channel on a tiny (EPC, N) row on ACT, not per tile.
"""

import sys
import types

import numpy as np

B, C, N, E = 2, 64, 1024, 32
NCORES = 8
EPC = B * E // NCORES  # 8 channels per core
P = 128
NB = N // P  # 8 row blocks

_CACHE = {}


def _ensure_hook_shim():
    """Make trace=True safe even when antenv.axon_hooks is absent."""
    try:
        import antenv.axon_hooks  # noqa: F401
    except ImportError:
        mod = types.ModuleType("antenv.axon_hooks")
        mod.get_axon_ntff_profile_hook = lambda: None
        mod.set_axon_ntff_profile_hook = lambda h: None
        sys.modules["antenv.axon_hooks"] = mod


def _register_gated_maxmul():
    """Register the fused out = max(in0,s0)*max(in1,s1) custom DVE op."""
    import concourse.dve_ops as dve_ops
    from concourse.dve_ops import DveOp, OPS, has_src1
    from concourse.dve_spec import C0, C1, Spec, Src0, Src1, lower, maxx
    from concourse.dve_uop import DveOpSpec

    for op in OPS:
        if op.name == "GATED_MAXMUL_ANT":
            return op

    spec = Spec(
        body=maxx(Src0, C0) * maxx(Src1, C1),
        reference=lambda in0, in1, s0, s1, imm2: np.maximum(in0, s0)
        * np.maximum(in1, s1),
    )
    op = DveOp("GATED_MAXMUL_ANT", spec, subdim=False, uops_sha={})
    OPS.append(op)
    # Rebuild the registry views that were snapshotted at import time.
    dve_ops.CUSTOM_DVE_SPECS[op.name] = op.spec
    opcode = dve_ops._CUSTOM_DVE_ROW_BASE + len(OPS) - 1
    assert opcode < 0x20
    dve_ops._SUB_OPCODE_FOR_NAME[op.name] = opcode
    # Pin the sha self-consistently (computed exactly as compile() does).
    for ver in ("v3", "v4"):
        s = DveOpSpec(
            name=op.name, opcode=opcode, uops=lower(spec, ver=ver),
            rd1_en=has_src1(spec),
        )
        op.uops_sha[ver] = s.sha(ver)
    return op


def _build_program():
    import concourse.bacc as bacc
    import concourse.mybir as mybir
    import concourse.tile as tile

    dt = mybir.dt.float32
    AF = mybir.ActivationFunctionType

    gated_op = _register_gated_maxmul()

    nc = bacc.Bacc("TRN2", target_bir_lowering=False, debug=False, num_devices=NCORES)

    emb = nc.declare_dram_parameter("emb", [C, N], dt, isOutput=False)
    w1t = nc.declare_dram_parameter("w1t", [C, EPC], dt, isOutput=False)
    w2t = nc.declare_dram_parameter("w2t", [C, EPC], dt, isOutput=False)
    th5c1 = nc.declare_dram_parameter("th5c1", [EPC, 1], dt, isOutput=False)
    th5c2 = nc.declare_dram_parameter("th5c2", [EPC, 1], dt, isOutput=False)
    eye = nc.declare_dram_parameter("eye", [P, P], dt, isOutput=False)
    out = nc.declare_dram_parameter("out", [EPC, N, N], dt, isOutput=True)

    H = N // 2  # matmul moving free-dim limit is 512

    with tile.TileContext(nc, pool_alloc_mode="queue") as tc:
        with (
            tc.tile_pool(name="const", bufs=1) as cpool,
            tc.tile_pool(name="rows", bufs=1) as rpool,
        ):
            sb_emb = cpool.tile([C, N], dt)
            nc.sync.dma_start(out=sb_emb[:], in_=emb[:])
            sb_w1t = cpool.tile([C, EPC], dt)
            nc.sync.dma_start(out=sb_w1t[:], in_=w1t[:])
            sb_w2t = cpool.tile([C, EPC], dt)
            nc.sync.dma_start(out=sb_w2t[:], in_=w2t[:])
            sb_th5c1 = cpool.tile([EPC, 1], dt)
            nc.sync.dma_start(out=sb_th5c1[:], in_=th5c1[:])
            sb_th5c2 = cpool.tile([EPC, 1], dt)
            nc.sync.dma_start(out=sb_th5c2[:], in_=th5c2[:])
            sb_eye = cpool.tile([P, P], dt)
            nc.sync.dma_start(out=sb_eye[:], in_=eye[:])
            sb_ones = cpool.tile([1, P], dt)
            nc.vector.memset(sb_ones[:], 1.0)

            # Row-layout intermediates (channel on partition, node on free).
            sb_vrow = rpool.tile([EPC, N], dt)   # 2*relu(t1)
            sb_grow = rpool.tile([EPC, N], dt)   # sigmoid(2*relu(t2))
            sb_dtrue = rpool.tile([EPC, N], dt)  # true diagonal values
            # Column layouts: [p, r*EPC + ch] = value at node r*128+p.
            sb_vcol = rpool.tile([P, NB * EPC], dt)
            sb_gcol = rpool.tile([P, NB * EPC], dt)
            sb_dcol = rpool.tile([P, NB * EPC], dt)

            with (
                tc.tile_pool(name="ph1ps", bufs=1, space="PSUM") as p1ps,
                tc.tile_pool(name="ph1sb", bufs=1) as p1sb,
            ):
                ps_t1 = p1ps.tile([EPC, N], dt)
                ps_t2 = p1ps.tile([EPC, N], dt)
                for h in range(2):
                    nc.tensor.matmul(
                        ps_t1[:, h * H:(h + 1) * H],
                        lhsT=sb_w1t[:],
                        rhs=sb_emb[:, h * H:(h + 1) * H],
                        start=True,
                        stop=True,
                    )
                    nc.tensor.matmul(
                        ps_t2[:, h * H:(h + 1) * H],
                        lhsT=sb_w2t[:],
                        rhs=sb_emb[:, h * H:(h + 1) * H],
                        start=True,
                        stop=True,
                    )
                nc.scalar.activation(sb_vrow[:], ps_t1[:], AF.Relu, scale=2.0)
                sb_urow = p1sb.tile([EPC, N], dt)
                nc.scalar.activation(sb_urow[:], ps_t2[:], AF.Relu, scale=2.0)
                nc.scalar.activation(sb_grow[:], sb_urow[:], AF.Sigmoid)
                # True diagonal: relu(2t1+th5_1) * sigmoid(relu(2t2+th5_2))
                sb_d1 = p1sb.tile([EPC, N], dt)
                nc.scalar.activation(
                    sb_d1[:], ps_t1[:], AF.Relu, bias=sb_th5c1[:], scale=2.0
                )
                sb_d2 = p1sb.tile([EPC, N], dt)
                nc.scalar.activation(
                    sb_d2[:], ps_t2[:], AF.Relu, bias=sb_th5c2[:], scale=2.0
                )
                nc.scalar.activation(sb_d2[:], sb_d2[:], AF.Sigmoid)
                nc.vector.tensor_mul(sb_dtrue[:], sb_d1[:], sb_d2[:])

            with (
                tc.tile_pool(name="colps", bufs=2, space="PSUM") as cps,
                tc.tile_pool(name="colsb", bufs=4) as csb,
            ):
                # v/g columns straight from emb: t_col[r] = emb_blk.T @ w
                # (independent of phase-1 rows, so it fills the pipeline
                # head); dcol still transposes the dtrue row.
                for r in range(NB):
                    pv = cps.tile([P, EPC], dt, tag="pv")
                    nc.tensor.matmul(
                        pv[:], lhsT=sb_emb[:, r * P:(r + 1) * P], rhs=sb_w1t[:],
                        start=True, stop=True,
                    )
                    nc.scalar.activation(
                        sb_vcol[:, r * EPC:(r + 1) * EPC], pv[:], AF.Relu, scale=2.0
                    )
                    pg = cps.tile([P, EPC], dt, tag="pg")
                    nc.tensor.matmul(
                        pg[:], lhsT=sb_emb[:, r * P:(r + 1) * P], rhs=sb_w2t[:],
                        start=True, stop=True,
                    )
                    ug = csb.tile([P, EPC], dt, tag="ug")
                    nc.scalar.activation(ug[:], pg[:], AF.Relu, scale=2.0)
                    nc.scalar.activation(
                        sb_gcol[:, r * EPC:(r + 1) * EPC], ug[:], AF.Sigmoid
                    )
                    pt_c = cps.tile([P, EPC], dt, tag="pt_c")
                    nc.tensor.transpose(
                        pt_c[:], sb_dtrue[:, r * P:(r + 1) * P], sb_eye[:EPC, :EPC]
                    )
                    nc.scalar.copy(sb_dcol[:, r * EPC:(r + 1) * EPC], pt_c[:])

            with (
                tc.tile_pool(name="jrepps", bufs=2, space="PSUM") as jps,
                tc.tile_pool(name="jrepsb", bufs=3) as jsb,
                tc.tile_pool(name="work", bufs=6) as wp,
            ):
                for ch in range(EPC):
                    # PE needs base partition 0 for both matmul operands;
                    # stage this channel's v/g row on partition 0 via DMA,
                    # then replicate across partitions with K=1 matmuls.
                    sb_vflat = jsb.tile([1, N], dt, tag="sb_vflat")
                    nc.sync.dma_start(out=sb_vflat[:], in_=sb_vrow[ch:ch + 1, :])
                    sb_gflat = jsb.tile([1, N], dt, tag="sb_gflat")
                    nc.sync.dma_start(out=sb_gflat[:], in_=sb_grow[ch:ch + 1, :])
                    ps_v = jps.tile([P, N], dt, tag="ps_v")
                    ps_g = jps.tile([P, N], dt, tag="ps_g")
                    for h in range(2):
                        nc.tensor.matmul(
                            ps_v[:, h * H:(h + 1) * H],
                            lhsT=sb_ones[:],
                            rhs=sb_vflat[:, h * H:(h + 1) * H],
                            start=True,
                            stop=True,
                        )
                        nc.tensor.matmul(
                            ps_g[:, h * H:(h + 1) * H],
                            lhsT=sb_ones[:],
                            rhs=sb_gflat[:, h * H:(h + 1) * H],
                            start=True,
                            stop=True,
                        )
                    sb_vj = jsb.tile([P, N], dt, tag="sb_vj")
                    nc.scalar.copy(sb_vj[:], ps_v[:])
                    sb_gj = jsb.tile([P, N], dt, tag="sb_gj")
                    nc.scalar.copy(sb_gj[:], ps_g[:])

                    for r in range(NB):
                        cb = r * P
                        ci = r * EPC + ch
                        o = wp.tile([P, N], dt, tag="o")
                        nc.vector._custom_dve(
                            gated_op,
                            out=o[:],
                            in0=sb_vj[:],
                            in1=sb_gj[:],
                            s0=sb_vcol[:, ci:ci + 1],
                            s1=sb_gcol[:, ci:ci + 1],
                        )
                        nc.vector.copy_predicated(
                            o[:, cb:cb + P],
                            sb_eye[:].bitcast(mybir.dt.int32),
                            sb_dcol[:, ci:ci + 1].broadcast_to([P, P]),
                        )
                        nc.sync.dma_start(out=out[ch, cb:cb + P, :], in_=o[:])

    nc.compile()
    return nc


def _get_program():
    if "nc" not in _CACHE:
        _CACHE["nc"] = _build_program()
    return _CACHE["nc"]


def kernel(**inputs):
    _ensure_hook_shim()
    from concourse.bass_utils import run_bass_kernel_spmd

    emb = np.ascontiguousarray(np.asarray(inputs["emb"], dtype=np.float32))
    th12_1 = np.asarray(inputs["th12_1"], dtype=np.float32)
    th12_2 = np.asarray(inputs["th12_2"], dtype=np.float32)
    th5_1 = np.asarray(inputs["th5_1"], dtype=np.float32)
    th5_2 = np.asarray(inputs["th5_2"], dtype=np.float32)
    eye = np.eye(P, dtype=np.float32)

    in_maps = []
    for k in range(NCORES):
        b = k // (NCORES // B)
        e0 = (k % (NCORES // B)) * EPC
        in_maps.append(
            {
                "emb": np.ascontiguousarray(emb[b]),
                "w1t": np.ascontiguousarray(th12_1[e0:e0 + EPC].T),
                "w2t": np.ascontiguousarray(th12_2[e0:e0 + EPC].T),
                "th5c1": np.ascontiguousarray(th5_1[e0:e0 + EPC, None]),
                "th5c2": np.ascontiguousarray(th5_2[e0:e0 + EPC, None]),
                "eye": eye,
            }
        )

    nc = _get_program()
    res = run_bass_kernel_spmd(nc, in_maps, core_ids=list(range(NCORES)))
    _CACHE["last_result"] = res

    out = np.empty((B, E, N, N), dtype=np.float32)
    for k in range(NCORES):
        b = k // (NCORES // B)
        e0 = (k % (NCORES // B)) * EPC
        out[b, e0:e0 + EPC] = res.results[k]["out"]
    return out



# revision 2
# speedup vs baseline: 1.1595x; 1.1595x over previous
"""Trainium2 Bass/Tile kernel for the GatedNode2Edge op.

Computes, for emb (B,C,N), th12_* (E,C), th5_* (E,):
    t_k  = th12_k @ emb[b]                      (E,N)
    m_k  = max(t_k[:,i], t_k[:,j]) pairwise     (E,N,N)
    adj  = relu(2*m_1 + th5_1*I)
    gate = sigmoid(relu(2*m_2 + th5_2*I))
    out  = adj * gate                           (B,E,N,N)

Sharding: the 64 (b,e) channels are split 8-per-core across 8 NeuronCores.

Math restructuring (off-diagonal):
    relu(2*max(a,b)) = max(2*relu(a), 2*relu(b))           (relu monotone)
    sigmoid(max(x,y)) = max(sigmoid(x), sigmoid(y))        (sigmoid monotone)
so with row vectors v = 2*relu(t1), g = sigmoid(2*relu(t2)):
    out[i,j] = max(v_i, v_j) * max(g_i, g_j)
one fused custom-DVE op per [128, N] output tile:
    out = maxx(Src0, C0) * maxx(Src1, C1)
with Src0 = v broadcast across partitions (PE outer-product), C0 = v column
slice (per-partition scalar), likewise Src1/C1 for g.

Performance structure vs the f32 baseline:
  - All pairwise tiles are fp16: halves both the DVE stream work and the
    HBM output traffic (the dominant cost). Host upcasts to f32.
  - The custom DVE op carries a hand-authored 2X_1PORT uop program
    (reads packed SRC/SRC_HI fp16 pairs, writes WR0_LO/WR0_HI), and each
    instruction sets perf_max=1 so the engine engages 2x mode: 2 output
    elements/cycle instead of 1.
  - The true diagonal is patched with ONE strided copy_predicated per
    channel (free AP [(8 blocks, 1152), (128, 1)]) instead of 8 per-tile
    patches.
  - Output leaves per channel as a single 2 MB DMA ([128, 8192] SBUF ->
    [8x128, 1024] DRAM 3D AP) for near-line-rate HBM writes.
"""

import sys
import types

import numpy as np

B, C, N, E = 2, 64, 1024, 32
NCORES = 8
EPC = B * E // NCORES  # 8 channels per core
P = 128
NB = N // P  # 8 row blocks
H = N // 2  # matmul moving free-dim limit is 512

PERF2X = True  # set False to fall back to 1x custom-DVE mode

_CACHE = {}


def _ensure_hook_shim():
    """Make trace=True safe even when antenv.axon_hooks is absent."""
    try:
        import antenv.axon_hooks  # noqa: F401
    except ImportError:
        mod = types.ModuleType("antenv.axon_hooks")
        mod.get_axon_ntff_profile_hook = lambda: None
        mod.set_axon_ntff_profile_hook = lambda h: None
        sys.modules["antenv.axon_hooks"] = mod


def _build_2x_uop():
    """2X_1PORT program for out = max(Src0,C0)*max(Src1,C1).

    Per cycle the engine delivers packed fp16 pairs: (SRC_0, SRC_0_HI) from
    rd0 and (SRC_1, SRC_1_HI) from rd1.  Six ALU blocks compute both
    results; the even (lo) result rides delay chain 0 to the writeback,
    the odd (hi) result arrives through the block-7 ALU bypass.
    """
    from concourse.dve_uop import (
        AluInp,
        AluOp,
        DelayInp,
        InpSel,
        OutPath,
        OutSel,
        Trigger,
        UopConfig,
    )

    u = UopConfig()
    # input lanes 1..6 feed block 0's delay chains 0..5
    u.enable_input(InpSel.SRC_0, 1)      # chain 0
    u.enable_input(InpSel.CONST_0, 2)    # chain 1
    u.enable_input(InpSel.SRC_1, 3)      # chain 2
    u.enable_input(InpSel.CONST_1, 4)    # chain 3
    u.enable_input(InpSel.SRC_0_HI, 5)   # chain 4
    u.enable_input(InpSel.SRC_1_HI, 6)   # chain 5
    dp = u.datapath_config
    # b0: m0 = max(S0, C0); latch all operands into the delay chains
    dp[0].enable_alu(AluOp.MAX, AluInp.PREV_DELAY_0, AluInp.PREV_DELAY_1)
    dp[0].pass_through_delay(1, 2, 3, 4, 5)
    # b1: m1 = max(S1, C1); chain0 <- m0
    dp[1].enable_alu(AluOp.MAX, AluInp.PREV_DELAY_2, AluInp.PREV_DELAY_3)
    dp[1].enable_delay_from_src(DelayInp.PREV_ALU_OUT, 0)
    dp[1].pass_through_delay(1, 3, 4, 5)
    # b2: r_lo = m1 * m0
    dp[2].enable_alu(AluOp.MULTIPLY, AluInp.PREV_ALU_OUT, AluInp.PREV_DELAY_0)
    dp[2].pass_through_delay(1, 3, 4, 5)
    # b3: m2 = max(S0_HI, C0); chain0 <- r_lo
    dp[3].enable_alu(AluOp.MAX, AluInp.PREV_DELAY_4, AluInp.PREV_DELAY_1)
    dp[3].enable_delay_from_src(DelayInp.PREV_ALU_OUT, 0)
    dp[3].pass_through_delay(3, 5)
    # b4: m3 = max(S1_HI, C1); chain2 <- m2; carry r_lo
    dp[4].enable_alu(AluOp.MAX, AluInp.PREV_DELAY_5, AluInp.PREV_DELAY_3)
    dp[4].enable_delay_from_src(DelayInp.PREV_ALU_OUT, 2)
    dp[4].pass_through_delay(0)
    # b5: r_hi = m3 * m2; carry r_lo
    dp[5].enable_alu(AluOp.MULTIPLY, AluInp.PREV_ALU_OUT, AluInp.PREV_DELAY_2)
    dp[5].pass_through_delay(0)
    # b6/b7: bypass r_hi down the pipe; carry r_lo
    dp[6].pass_through_alu()
    dp[6].pass_through_delay(0)
    dp[7].pass_through_alu()
    dp[7].pass_through_delay(0)
    u.enable_output(OutSel.DELAY_0, OutPath.WR0_LO)
    u.enable_output(OutSel.ALU_OUT, OutPath.WR0_HI)
    u.require_inp0 = 1
    u.require_inp1 = 1
    u.trigger = (Trigger.SRC_TENSOR_DONE, Trigger.NONE, Trigger.NONE)
    return u


def _register_gated_maxmul():
    """Register the fused out = max(in0,s0)*max(in1,s1) custom DVE op,
    with both the compiler-lowered 1x program and the hand-built 2x one."""
    import concourse.dve_ops as dve_ops
    from concourse.dve_ops import _COMPILE_CACHE, OPS, DveOp, has_src1
    from concourse.dve_spec import C0, C1, Spec, Src0, Src1, lower, maxx
    from concourse.dve_uop import DveOpSpec

    NAME = "GATED_MAXMUL2X_ANT"
    for op in OPS:
        if op.name == NAME:
            return op

    spec = Spec(
        body=maxx(Src0, C0) * maxx(Src1, C1),
        reference=lambda in0, in1, s0, s1, imm2: np.maximum(in0, s0)
        * np.maximum(in1, s1),
    )
    op = DveOp(NAME, spec, subdim=False, uops_sha={})
    OPS.append(op)
    # Rebuild the registry views that were snapshotted at import time.
    dve_ops.CUSTOM_DVE_SPECS[op.name] = op.spec
    opcode = dve_ops._CUSTOM_DVE_ROW_BASE + len(OPS) - 1
    assert opcode < 0x20
    dve_ops._SUB_OPCODE_FOR_NAME[op.name] = opcode
    # TRN2 is DVE ver v3. Pre-seed the compile cache with a spec carrying
    # the 2x program; pin the sha self-consistently.
    s = DveOpSpec(
        name=op.name,
        opcode=opcode,
        uops=lower(spec, ver="v3"),
        uops_2x=[_build_2x_uop()] if PERF2X else None,
        perf_max=1 if PERF2X else 0,
        rd1_en=has_src1(spec),
    )
    op.uops_sha["v3"] = s.sha("v3")
    _COMPILE_CACHE[(op.name, "v3")] = s
    return op


def _build_program():
    import concourse.bacc as bacc
    import concourse.mybir as mybir
    import concourse.tile as tile
    from concourse.ap import AP

    dt = mybir.dt.float32
    f16 = mybir.dt.float16
    AF = mybir.ActivationFunctionType

    gated_op = _register_gated_maxmul()

    nc = bacc.Bacc("TRN2", target_bir_lowering=False, debug=False, num_devices=NCORES)

    emb = nc.declare_dram_parameter("emb", [C, N], dt, isOutput=False)
    w1t = nc.declare_dram_parameter("w1t", [C, EPC], dt, isOutput=False)
    w2t = nc.declare_dram_parameter("w2t", [C, EPC], dt, isOutput=False)
    th5c1 = nc.declare_dram_parameter("th5c1", [EPC, 1], dt, isOutput=False)
    th5c2 = nc.declare_dram_parameter("th5c2", [EPC, 1], dt, isOutput=False)
    eye = nc.declare_dram_parameter("eye", [P, P], dt, isOutput=False)
    eye16 = nc.declare_dram_parameter("eye16", [P, P], f16, isOutput=False)
    ones16 = nc.declare_dram_parameter("ones16", [1, P], f16, isOutput=False)
    out = nc.declare_dram_parameter("out", [EPC, N, N], f16, isOutput=True)

    with tile.TileContext(nc, pool_alloc_mode="queue") as tc:
        with (
            tc.tile_pool(name="const", bufs=1) as cpool,
            tc.tile_pool(name="rows", bufs=1) as rpool,
        ):
            sb_emb = cpool.tile([C, N], dt)
            nc.sync.dma_start(out=sb_emb[:], in_=emb[:])
            sb_w1t = cpool.tile([C, EPC], dt)
            nc.sync.dma_start(out=sb_w1t[:], in_=w1t[:])
            sb_w2t = cpool.tile([C, EPC], dt)
            nc.sync.dma_start(out=sb_w2t[:], in_=w2t[:])
            sb_th5c1 = cpool.tile([EPC, 1], dt)
            nc.sync.dma_start(out=sb_th5c1[:], in_=th5c1[:])
            sb_th5c2 = cpool.tile([EPC, 1], dt)
            nc.sync.dma_start(out=sb_th5c2[:], in_=th5c2[:])
            sb_eye = cpool.tile([P, P], dt)
            nc.sync.dma_start(out=sb_eye[:], in_=eye[:])
            sb_eye16 = cpool.tile([P, P], f16)
            nc.sync.dma_start(out=sb_eye16[:], in_=eye16[:])
            sb_ones16 = cpool.tile([1, P], f16)
            nc.sync.dma_start(out=sb_ones16[:], in_=ones16[:])

            # Row-layout intermediates (channel on partition, node on free).
            # v rows in [:, :N], g rows in [:, N:] so one region feeds the
            # per-channel staging slices.
            sb_vg16 = rpool.tile([EPC, 2 * N], f16)
            sb_dtrue = rpool.tile([EPC, N], dt)  # true diagonal values, f32
            # Column layouts: [p, r*EPC + ch] = value at node r*128+p.
            sb_vcol = rpool.tile([P, NB * EPC], dt)
            sb_gcol = rpool.tile([P, NB * EPC], dt)
            sb_dcol16 = rpool.tile([P, NB * EPC], f16)
            # Staging row: all channels' v|g rows flattened onto partition 0.
            sb_flat = rpool.tile([1, EPC * 2 * N], f16)

            with (
                tc.tile_pool(name="ph1ps", bufs=1, space="PSUM") as p1ps,
                tc.tile_pool(name="ph1sb", bufs=1) as p1sb,
            ):
                ps_t1 = p1ps.tile([EPC, N], dt)
                ps_t2 = p1ps.tile([EPC, N], dt)
                for h in range(2):
                    nc.tensor.matmul(
                        ps_t1[:, h * H:(h + 1) * H],
                        lhsT=sb_w1t[:],
                        rhs=sb_emb[:, h * H:(h + 1) * H],
                        start=True,
                        stop=True,
                    )
                    nc.tensor.matmul(
                        ps_t2[:, h * H:(h + 1) * H],
                        lhsT=sb_w2t[:],
                        rhs=sb_emb[:, h * H:(h + 1) * H],
                        start=True,
                        stop=True,
                    )
                nc.scalar.activation(sb_vg16[:, :N], ps_t1[:], AF.Relu, scale=2.0)
                sb_urow = p1sb.tile([EPC, N], dt)
                nc.scalar.activation(sb_urow[:], ps_t2[:], AF.Relu, scale=2.0)
                nc.scalar.activation(sb_vg16[:, N:], sb_urow[:], AF.Sigmoid)
                # True diagonal: relu(2t1+th5_1) * sigmoid(relu(2t2+th5_2))
                sb_d1 = p1sb.tile([EPC, N], dt)
                nc.scalar.activation(
                    sb_d1[:], ps_t1[:], AF.Relu, bias=sb_th5c1[:], scale=2.0
                )
                sb_d2 = p1sb.tile([EPC, N], dt)
                nc.scalar.activation(
                    sb_d2[:], ps_t2[:], AF.Relu, bias=sb_th5c2[:], scale=2.0
                )
                nc.scalar.activation(sb_d2[:], sb_d2[:], AF.Sigmoid)
                nc.vector.tensor_mul(sb_dtrue[:], sb_d1[:], sb_d2[:])

            with (
                tc.tile_pool(name="colps", bufs=2, space="PSUM") as cps,
                tc.tile_pool(name="colsb", bufs=4) as csb,
            ):
                # v/g columns straight from emb: t_col[r] = emb_blk.T @ w
                # (independent of phase-1 rows, so it fills the pipeline
                # head); dcol still transposes the dtrue row.
                for r in range(NB):
                    pv = cps.tile([P, EPC], dt, tag="pv")
                    nc.tensor.matmul(
                        pv[:], lhsT=sb_emb[:, r * P:(r + 1) * P], rhs=sb_w1t[:],
                        start=True, stop=True,
                    )
                    nc.scalar.activation(
                        sb_vcol[:, r * EPC:(r + 1) * EPC], pv[:], AF.Relu, scale=2.0
                    )
                    pg = cps.tile([P, EPC], dt, tag="pg")
                    nc.tensor.matmul(
                        pg[:], lhsT=sb_emb[:, r * P:(r + 1) * P], rhs=sb_w2t[:],
                        start=True, stop=True,
                    )
                    ug = csb.tile([P, EPC], dt, tag="ug")
                    nc.scalar.activation(ug[:], pg[:], AF.Relu, scale=2.0)
                    nc.scalar.activation(
                        sb_gcol[:, r * EPC:(r + 1) * EPC], ug[:], AF.Sigmoid
                    )
                    pt_c = cps.tile([P, EPC], dt, tag="pt_c")
                    nc.tensor.transpose(
                        pt_c[:], sb_dtrue[:, r * P:(r + 1) * P], sb_eye[:EPC, :EPC]
                    )
                    nc.scalar.copy(sb_dcol16[:, r * EPC:(r + 1) * EPC], pt_c[:])

            with (
                tc.tile_pool(name="jrepps", bufs=2, space="PSUM") as jps,
                tc.tile_pool(name="jrepsb", bufs=3) as jsb,
                tc.tile_pool(name="obuf", bufs=2) as opool,
            ):
                eye_ap = sb_eye16[:]
                dcol_ap = sb_dcol16[:]
                for ch in range(EPC):
                    # PE needs base partition 0 for both matmul operands;
                    # stage this channel's v|g row pair on partition 0,
                    # then replicate across partitions with K=1 matmuls.
                    fo = ch * 2 * N
                    nc.sync.dma_start(
                        out=sb_flat[0:1, fo:fo + 2 * N], in_=sb_vg16[ch:ch + 1, :]
                    )
                    ps_v = jps.tile([P, N], dt, tag="ps_v")
                    ps_g = jps.tile([P, N], dt, tag="ps_g")
                    for h in range(2):
                        nc.tensor.matmul(
                            ps_v[:, h * H:(h + 1) * H],
                            lhsT=sb_ones16[:],
                            rhs=sb_flat[0:1, fo + h * H:fo + (h + 1) * H],
                            start=True,
                            stop=True,
                        )
                        nc.tensor.matmul(
                            ps_g[:, h * H:(h + 1) * H],
                            lhsT=sb_ones16[:],
                            rhs=sb_flat[0:1, fo + N + h * H:fo + N + (h + 1) * H],
                            start=True,
                            stop=True,
                        )
                    sb_vj = jsb.tile([P, N], f16, tag="sb_vj")
                    nc.scalar.copy(sb_vj[:], ps_v[:])
                    sb_gj = jsb.tile([P, N], f16, tag="sb_gj")
                    nc.scalar.copy(sb_gj[:], ps_g[:])

                    o = opool.tile([P, NB * N], f16, tag="o")
                    for r in range(NB):
                        ci = r * EPC + ch
                        inst = nc.vector._custom_dve(
                            gated_op,
                            out=o[:, r * N:(r + 1) * N],
                            in0=sb_vj[:],
                            in1=sb_gj[:],
                            s0=sb_vcol[:, ci:ci + 1],
                            s1=sb_gcol[:, ci:ci + 1],
                        )
                        if PERF2X:
                            inst.perf_max = 1

                    # One strided patch for the channel's 8 diagonal blocks:
                    # block r's diagonal segment lives at free offset
                    # r*N + r*P, length P.
                    o_ap = o[:]
                    patch_out = AP(
                        o_ap.tensor,
                        o_ap.offset,
                        [list(o_ap.ap[0]), [N + P, NB], [1, P]],
                    )
                    patch_mask = AP(
                        eye_ap.tensor,
                        eye_ap.offset,
                        [list(eye_ap.ap[0]), [0, NB], [1, P]],
                    ).bitcast(mybir.dt.int16)
                    patch_data = AP(
                        dcol_ap.tensor,
                        dcol_ap.offset + ch,
                        [list(dcol_ap.ap[0]), [EPC, NB], [0, P]],
                    )
                    nc.vector.copy_predicated(patch_out, patch_mask, patch_data)

                    # Whole channel leaves in one 2 MB DMA:
                    # SBUF [p, r*N + j] -> DRAM [ch, r*P + p, j].
                    src = AP(
                        o_ap.tensor,
                        o_ap.offset,
                        [list(o_ap.ap[0]), [N, NB], [1, N]],
                    )
                    out_ap = out[:]
                    dst = AP(
                        out_ap.tensor,
                        ch * N * N,
                        [[N, P], [P * N, NB], [1, N]],
                    )
                    nc.sync.dma_start(out=dst, in_=src)

    nc.compile()
    return nc


def _get_program():
    if "nc" not in _CACHE:
        _CACHE["nc"] = _build_program()
    return _CACHE["nc"]


def kernel(**inputs):
    _ensure_hook_shim()
    from concourse.bass_utils import run_bass_kernel_spmd

    emb = np.ascontiguousarray(np.asarray(inputs["emb"], dtype=np.float32))
    th12_1 = np.asarray(inputs["th12_1"], dtype=np.float32)
    th12_2 = np.asarray(inputs["th12_2"], dtype=np.float32)
    th5_1 = np.asarray(inputs["th5_1"], dtype=np.float32)
    th5_2 = np.asarray(inputs["th5_2"], dtype=np.float32)
    eye = np.eye(P, dtype=np.float32)
    eye16 = np.eye(P, dtype=np.float16)
    ones16 = np.ones((1, P), dtype=np.float16)

    in_maps = []
    for k in range(NCORES):
        b = k // (NCORES // B)
        e0 = (k % (NCORES // B)) * EPC
        in_maps.append(
            {
                "emb": np.ascontiguousarray(emb[b]),
                "w1t": np.ascontiguousarray(th12_1[e0:e0 + EPC].T),
                "w2t": np.ascontiguousarray(th12_2[e0:e0 + EPC].T),
                "th5c1": np.ascontiguousarray(th5_1[e0:e0 + EPC, None]),
                "th5c2": np.ascontiguousarray(th5_2[e0:e0 + EPC, None]),
                "eye": eye,
                "eye16": eye16,
                "ones16": ones16,
            }
        )

    nc = _get_program()
    res = run_bass_kernel_spmd(nc, in_maps, core_ids=list(range(NCORES)))
    _CACHE["last_result"] = res

    out = np.empty((B, E, N, N), dtype=np.float32)
    for k in range(NCORES):
        b = k // (NCORES // B)
        e0 = (k % (NCORES // B)) * EPC
        out[b, e0:e0 + EPC] = res.results[k]["out"].astype(np.float32)
    return out


# revision 3
# speedup vs baseline: 1.4857x; 1.2813x over previous
"""Trainium2 Bass/Tile kernel for the GatedNode2Edge op.

Computes, for emb (B,C,N), th12_* (E,C), th5_* (E,):
    t_k  = th12_k @ emb[b]                      (E,N)
    m_k  = max(t_k[:,i], t_k[:,j]) pairwise     (E,N,N)
    adj  = relu(2*m_1 + th5_1*I)
    gate = sigmoid(relu(2*m_2 + th5_2*I))
    out  = adj * gate                           (B,E,N,N)

Sharding: the 64 (b,e) channels are split 8-per-core across 8 NeuronCores.

Math restructuring (off-diagonal):
    relu(2*max(a,b)) = max(2*relu(a), 2*relu(b))           (relu monotone)
    sigmoid(max(x,y)) = max(sigmoid(x), sigmoid(y))        (sigmoid monotone)
so with row vectors v = 2*relu(t1), g = sigmoid(2*relu(t2)):
    out[i,j] = max(v_i, v_j) * max(g_i, g_j)
one fused custom-DVE op per [128, N] output tile:
    out = maxx(Src0, C0) * maxx(Src1, C1)
with Src0 = v broadcast across partitions (PE outer-product), C0 = v column
slice (per-partition scalar), likewise Src1/C1 for g.

Performance structure vs the f32 baseline:
  - All pairwise tiles are fp16: halves both the DVE stream work and the
    HBM output traffic (the dominant cost). Host upcasts to f32.
  - The custom DVE op carries a hand-authored 2X_1PORT uop program
    (reads packed SRC/SRC_HI fp16 pairs, writes WR0_LO/WR0_HI), and each
    instruction sets perf_max=1 so the engine engages 2x mode: 2 output
    elements/cycle instead of 1.
  - The true diagonal is patched with ONE strided copy_predicated per
    channel (free AP [(8 blocks, 1152), (128, 1)]) instead of 8 per-tile
    patches.
  - Output leaves per channel as a single 2 MB DMA ([128, 8192] SBUF ->
    [8x128, 1024] DRAM 3D AP) for near-line-rate HBM writes.
"""

import sys
import types

import numpy as np

B, C, N, E = 2, 64, 1024, 32
NCORES = 8
EPC = B * E // NCORES  # 8 channels per core
P = 128
NB = N // P  # 8 row blocks
H = N // 2  # matmul moving free-dim limit is 512

PERF2X = True  # set False to fall back to 1x custom-DVE mode

_CACHE = {}


def _ensure_hook_shim():
    """Make trace=True safe even when antenv.axon_hooks is absent."""
    try:
        import antenv.axon_hooks  # noqa: F401
    except ImportError:
        mod = types.ModuleType("antenv.axon_hooks")
        mod.get_axon_ntff_profile_hook = lambda: None
        mod.set_axon_ntff_profile_hook = lambda h: None
        sys.modules["antenv.axon_hooks"] = mod


def _build_2x_uop():
    """2X_1PORT program for out = max(Src0,C0)*max(Src1,C1).

    Per cycle the engine delivers packed fp16 pairs: (SRC_0, SRC_0_HI) from
    rd0 and (SRC_1, SRC_1_HI) from rd1.  Six ALU blocks compute both
    results; the even (lo) result rides delay chain 0 to the writeback,
    the odd (hi) result arrives through the block-7 ALU bypass.
    """
    from concourse.dve_uop import (
        AluInp,
        AluOp,
        DelayInp,
        InpSel,
        OutPath,
        OutSel,
        Trigger,
        UopConfig,
    )

    u = UopConfig()
    # input lanes 1..6 feed block 0's delay chains 0..5
    u.enable_input(InpSel.SRC_0, 1)      # chain 0
    u.enable_input(InpSel.CONST_0, 2)    # chain 1
    u.enable_input(InpSel.SRC_1, 3)      # chain 2
    u.enable_input(InpSel.CONST_1, 4)    # chain 3
    u.enable_input(InpSel.SRC_0_HI, 5)   # chain 4
    u.enable_input(InpSel.SRC_1_HI, 6)   # chain 5
    dp = u.datapath_config
    # b0: m0 = max(S0, C0); latch all operands into the delay chains
    dp[0].enable_alu(AluOp.MAX, AluInp.PREV_DELAY_0, AluInp.PREV_DELAY_1)
    dp[0].pass_through_delay(1, 2, 3, 4, 5)
    # b1: m1 = max(S1, C1); chain0 <- m0
    dp[1].enable_alu(AluOp.MAX, AluInp.PREV_DELAY_2, AluInp.PREV_DELAY_3)
    dp[1].enable_delay_from_src(DelayInp.PREV_ALU_OUT, 0)
    dp[1].pass_through_delay(1, 3, 4, 5)
    # b2: r_lo = m1 * m0
    dp[2].enable_alu(AluOp.MULTIPLY, AluInp.PREV_ALU_OUT, AluInp.PREV_DELAY_0)
    dp[2].pass_through_delay(1, 3, 4, 5)
    # b3: m2 = max(S0_HI, C0); chain0 <- r_lo
    dp[3].enable_alu(AluOp.MAX, AluInp.PREV_DELAY_4, AluInp.PREV_DELAY_1)
    dp[3].enable_delay_from_src(DelayInp.PREV_ALU_OUT, 0)
    dp[3].pass_through_delay(3, 5)
    # b4: m3 = max(S1_HI, C1); chain2 <- m2; carry r_lo
    dp[4].enable_alu(AluOp.MAX, AluInp.PREV_DELAY_5, AluInp.PREV_DELAY_3)
    dp[4].enable_delay_from_src(DelayInp.PREV_ALU_OUT, 2)
    dp[4].pass_through_delay(0)
    # b5: r_hi = m3 * m2; carry r_lo
    dp[5].enable_alu(AluOp.MULTIPLY, AluInp.PREV_ALU_OUT, AluInp.PREV_DELAY_2)
    dp[5].pass_through_delay(0)
    # b6/b7: bypass r_hi down the pipe; carry r_lo
    dp[6].pass_through_alu()
    dp[6].pass_through_delay(0)
    dp[7].pass_through_alu()
    dp[7].pass_through_delay(0)
    u.enable_output(OutSel.DELAY_0, OutPath.WR0_LO)
    u.enable_output(OutSel.ALU_OUT, OutPath.WR0_HI)
    u.require_inp0 = 1
    u.require_inp1 = 1
    u.trigger = (Trigger.SRC_TENSOR_DONE, Trigger.NONE, Trigger.NONE)
    return u


def _register_gated_maxmul():
    """Register the fused out = max(in0,s0)*max(in1,s1) custom DVE op,
    with both the compiler-lowered 1x program and the hand-built 2x one."""
    import concourse.dve_ops as dve_ops
    from concourse.dve_ops import _COMPILE_CACHE, OPS, DveOp, has_src1
    from concourse.dve_spec import C0, C1, Spec, Src0, Src1, lower, maxx
    from concourse.dve_uop import DveOpSpec

    NAME = "GATED_MAXMUL2X_ANT"
    for op in OPS:
        if op.name == NAME:
            return op

    spec = Spec(
        body=maxx(Src0, C0) * maxx(Src1, C1),
        reference=lambda in0, in1, s0, s1, imm2: np.maximum(in0, s0)
        * np.maximum(in1, s1),
    )
    op = DveOp(NAME, spec, subdim=False, uops_sha={})
    OPS.append(op)
    # Rebuild the registry views that were snapshotted at import time.
    dve_ops.CUSTOM_DVE_SPECS[op.name] = op.spec
    opcode = dve_ops._CUSTOM_DVE_ROW_BASE + len(OPS) - 1
    assert opcode < 0x20
    dve_ops._SUB_OPCODE_FOR_NAME[op.name] = opcode
    # TRN2 is DVE ver v3. Pre-seed the compile cache with a spec carrying
    # the 2x program; pin the sha self-consistently.
    s = DveOpSpec(
        name=op.name,
        opcode=opcode,
        uops=lower(spec, ver="v3"),
        uops_2x=[_build_2x_uop()] if PERF2X else None,
        perf_max=1 if PERF2X else 0,
        rd1_en=has_src1(spec),
    )
    op.uops_sha["v3"] = s.sha("v3")
    _COMPILE_CACHE[(op.name, "v3")] = s
    return op


def _build_program():
    import concourse.bacc as bacc
    import concourse.mybir as mybir
    import concourse.tile as tile
    from concourse.ap import AP

    dt = mybir.dt.float32
    f16 = mybir.dt.float16
    AF = mybir.ActivationFunctionType

    gated_op = _register_gated_maxmul()

    nc = bacc.Bacc("TRN2", target_bir_lowering=False, debug=False, num_devices=NCORES)

    emb = nc.declare_dram_parameter("emb", [C, N], dt, isOutput=False)
    w1t = nc.declare_dram_parameter("w1t", [C, EPC], dt, isOutput=False)
    w2t = nc.declare_dram_parameter("w2t", [C, EPC], dt, isOutput=False)
    th5c1 = nc.declare_dram_parameter("th5c1", [EPC, 1], dt, isOutput=False)
    th5c2 = nc.declare_dram_parameter("th5c2", [EPC, 1], dt, isOutput=False)
    eye = nc.declare_dram_parameter("eye", [P, P], dt, isOutput=False)
    eye16 = nc.declare_dram_parameter("eye16", [P, P], f16, isOutput=False)
    ones16 = nc.declare_dram_parameter("ones16", [1, P], f16, isOutput=False)
    out = nc.declare_dram_parameter("out", [EPC, N, N], f16, isOutput=True)

    with tile.TileContext(nc, pool_alloc_mode="queue") as tc:
        with (
            tc.tile_pool(name="const", bufs=1) as cpool,
            tc.tile_pool(name="rows", bufs=1) as rpool,
        ):
            sb_emb = cpool.tile([C, N], dt)
            nc.sync.dma_start(out=sb_emb[:], in_=emb[:])
            sb_w1t = cpool.tile([C, EPC], dt)
            nc.sync.dma_start(out=sb_w1t[:], in_=w1t[:])
            sb_w2t = cpool.tile([C, EPC], dt)
            nc.sync.dma_start(out=sb_w2t[:], in_=w2t[:])
            sb_th5c1 = cpool.tile([EPC, 1], dt)
            nc.sync.dma_start(out=sb_th5c1[:], in_=th5c1[:])
            sb_th5c2 = cpool.tile([EPC, 1], dt)
            nc.sync.dma_start(out=sb_th5c2[:], in_=th5c2[:])
            sb_eye = cpool.tile([P, P], dt)
            nc.sync.dma_start(out=sb_eye[:], in_=eye[:])
            sb_eye16 = cpool.tile([P, P], f16)
            nc.sync.dma_start(out=sb_eye16[:], in_=eye16[:])
            sb_ones16 = cpool.tile([1, P], f16)
            nc.sync.dma_start(out=sb_ones16[:], in_=ones16[:])

            # Row-layout intermediates (channel on partition, node on free).
            # v rows in [:, :N], g rows in [:, N:] so one region feeds the
            # per-channel staging slices.
            sb_vg16 = rpool.tile([EPC, 2 * N], f16)
            sb_dtrue = rpool.tile([EPC, N], dt)  # true diagonal values, f32
            # Column layouts: [p, r*EPC + ch] = value at node r*128+p.
            sb_vcol = rpool.tile([P, NB * EPC], dt)
            sb_gcol = rpool.tile([P, NB * EPC], dt)
            sb_dcol16 = rpool.tile([P, NB * EPC], f16)
            # Staging row: all channels' v|g rows flattened onto partition 0.
            sb_flat = rpool.tile([1, EPC * 2 * N], f16)

            with (
                tc.tile_pool(name="ph1ps", bufs=1, space="PSUM") as p1ps,
                tc.tile_pool(name="ph1sb", bufs=1) as p1sb,
            ):
                ps_t1 = p1ps.tile([EPC, N], dt)
                ps_t2 = p1ps.tile([EPC, N], dt)
                for h in range(2):
                    nc.tensor.matmul(
                        ps_t1[:, h * H:(h + 1) * H],
                        lhsT=sb_w1t[:],
                        rhs=sb_emb[:, h * H:(h + 1) * H],
                        start=True,
                        stop=True,
                    )
                    nc.tensor.matmul(
                        ps_t2[:, h * H:(h + 1) * H],
                        lhsT=sb_w2t[:],
                        rhs=sb_emb[:, h * H:(h + 1) * H],
                        start=True,
                        stop=True,
                    )
                nc.scalar.activation(sb_vg16[:, :N], ps_t1[:], AF.Relu, scale=2.0)
                sb_urow = p1sb.tile([EPC, N], dt)
                nc.scalar.activation(sb_urow[:], ps_t2[:], AF.Relu, scale=2.0)
                nc.scalar.activation(sb_vg16[:, N:], sb_urow[:], AF.Sigmoid)
                # True diagonal: relu(2t1+th5_1) * sigmoid(relu(2t2+th5_2))
                sb_d1 = p1sb.tile([EPC, N], dt)
                nc.scalar.activation(
                    sb_d1[:], ps_t1[:], AF.Relu, bias=sb_th5c1[:], scale=2.0
                )
                sb_d2 = p1sb.tile([EPC, N], dt)
                nc.scalar.activation(
                    sb_d2[:], ps_t2[:], AF.Relu, bias=sb_th5c2[:], scale=2.0
                )
                nc.scalar.activation(sb_d2[:], sb_d2[:], AF.Sigmoid)
                nc.vector.tensor_mul(sb_dtrue[:], sb_d1[:], sb_d2[:])

            with (
                tc.tile_pool(name="colps", bufs=2, space="PSUM") as cps,
                tc.tile_pool(name="colsb", bufs=4) as csb,
            ):
                # v/g columns straight from emb: t_col[r] = emb_blk.T @ w
                # (independent of phase-1 rows, so it fills the pipeline
                # head); dcol still transposes the dtrue row.
                for r in range(NB):
                    pv = cps.tile([P, EPC], dt, tag="pv")
                    nc.tensor.matmul(
                        pv[:], lhsT=sb_emb[:, r * P:(r + 1) * P], rhs=sb_w1t[:],
                        start=True, stop=True,
                    )
                    nc.scalar.activation(
                        sb_vcol[:, r * EPC:(r + 1) * EPC], pv[:], AF.Relu, scale=2.0
                    )
                    pg = cps.tile([P, EPC], dt, tag="pg")
                    nc.tensor.matmul(
                        pg[:], lhsT=sb_emb[:, r * P:(r + 1) * P], rhs=sb_w2t[:],
                        start=True, stop=True,
                    )
                    ug = csb.tile([P, EPC], dt, tag="ug")
                    nc.scalar.activation(ug[:], pg[:], AF.Relu, scale=2.0)
                    nc.scalar.activation(
                        sb_gcol[:, r * EPC:(r + 1) * EPC], ug[:], AF.Sigmoid
                    )
                    pt_c = cps.tile([P, EPC], dt, tag="pt_c")
                    nc.tensor.transpose(
                        pt_c[:], sb_dtrue[:, r * P:(r + 1) * P], sb_eye[:EPC, :EPC]
                    )
                    nc.scalar.copy(sb_dcol16[:, r * EPC:(r + 1) * EPC], pt_c[:])

            with (
                tc.tile_pool(name="jrepps", bufs=2, space="PSUM") as jps,
                tc.tile_pool(name="jrepsb", bufs=3) as jsb,
                tc.tile_pool(name="obuf", bufs=2) as opool,
            ):
                eye_ap = sb_eye16[:]
                dcol_ap = sb_dcol16[:]
                for ch in range(EPC):
                    # PE needs base partition 0 for both matmul operands;
                    # stage this channel's v|g row pair on partition 0,
                    # then replicate across partitions with K=1 matmuls.
                    fo = ch * 2 * N
                    nc.sync.dma_start(
                        out=sb_flat[0:1, fo:fo + 2 * N], in_=sb_vg16[ch:ch + 1, :]
                    )
                    ps_v = jps.tile([P, N], dt, tag="ps_v")
                    ps_g = jps.tile([P, N], dt, tag="ps_g")
                    for h in range(2):
                        nc.tensor.matmul(
                            ps_v[:, h * H:(h + 1) * H],
                            lhsT=sb_ones16[:],
                            rhs=sb_flat[0:1, fo + h * H:fo + (h + 1) * H],
                            start=True,
                            stop=True,
                        )
                        nc.tensor.matmul(
                            ps_g[:, h * H:(h + 1) * H],
                            lhsT=sb_ones16[:],
                            rhs=sb_flat[0:1, fo + N + h * H:fo + N + (h + 1) * H],
                            start=True,
                            stop=True,
                        )
                    sb_vj = jsb.tile([P, N], f16, tag="sb_vj")
                    nc.scalar.copy(sb_vj[:], ps_v[:])
                    sb_gj = jsb.tile([P, N], f16, tag="sb_gj")
                    nc.scalar.copy(sb_gj[:], ps_g[:])

                    o = opool.tile([P, NB * N], f16, tag="o")
                    for r in range(NB):
                        ci = r * EPC + ch
                        inst = nc.vector._custom_dve(
                            gated_op,
                            out=o[:, r * N:(r + 1) * N],
                            in0=sb_vj[:],
                            in1=sb_gj[:],
                            s0=sb_vcol[:, ci:ci + 1],
                            s1=sb_gcol[:, ci:ci + 1],
                        )
                        if PERF2X:
                            inst.ins.perf_max = 1

                    # One strided patch for the channel's 8 diagonal blocks:
                    # block r's diagonal segment lives at free offset
                    # r*N + r*P, length P.
                    o_ap = o[:]
                    patch_out = AP(
                        o_ap.tensor,
                        o_ap.offset,
                        [list(o_ap.ap[0]), [N + P, NB], [1, P]],
                    )
                    patch_mask = AP(
                        eye_ap.tensor,
                        eye_ap.offset,
                        [list(eye_ap.ap[0]), [0, NB], [1, P]],
                    ).bitcast(mybir.dt.int16)
                    patch_data = AP(
                        dcol_ap.tensor,
                        dcol_ap.offset + ch,
                        [list(dcol_ap.ap[0]), [EPC, NB], [0, P]],
                    )
                    nc.vector.copy_predicated(patch_out, patch_mask, patch_data)

                    # Whole channel leaves in one 2 MB DMA:
                    # SBUF [p, r*N + j] -> DRAM [ch, r*P + p, j].
                    src = AP(
                        o_ap.tensor,
                        o_ap.offset,
                        [list(o_ap.ap[0]), [N, NB], [1, N]],
                    )
                    out_ap = out[:]
                    dst = AP(
                        out_ap.tensor,
                        ch * N * N,
                        [[N, P], [P * N, NB], [1, N]],
                    )
                    nc.sync.dma_start(out=dst, in_=src)

    nc.compile()
    return nc


def _get_program():
    if "nc" not in _CACHE:
        _CACHE["nc"] = _build_program()
    return _CACHE["nc"]


def kernel(**inputs):
    _ensure_hook_shim()
    from concourse.bass_utils import run_bass_kernel_spmd

    emb = np.ascontiguousarray(np.asarray(inputs["emb"], dtype=np.float32))
    th12_1 = np.asarray(inputs["th12_1"], dtype=np.float32)
    th12_2 = np.asarray(inputs["th12_2"], dtype=np.float32)
    th5_1 = np.asarray(inputs["th5_1"], dtype=np.float32)
    th5_2 = np.asarray(inputs["th5_2"], dtype=np.float32)
    eye = np.eye(P, dtype=np.float32)
    eye16 = np.eye(P, dtype=np.float16)
    ones16 = np.ones((1, P), dtype=np.float16)

    in_maps = []
    for k in range(NCORES):
        b = k // (NCORES // B)
        e0 = (k % (NCORES // B)) * EPC
        in_maps.append(
            {
                "emb": np.ascontiguousarray(emb[b]),
                "w1t": np.ascontiguousarray(th12_1[e0:e0 + EPC].T),
                "w2t": np.ascontiguousarray(th12_2[e0:e0 + EPC].T),
                "th5c1": np.ascontiguousarray(th5_1[e0:e0 + EPC, None]),
                "th5c2": np.ascontiguousarray(th5_2[e0:e0 + EPC, None]),
                "eye": eye,
                "eye16": eye16,
                "ones16": ones16,
            }
        )

    nc = _get_program()
    res = run_bass_kernel_spmd(nc, in_maps, core_ids=list(range(NCORES)))
    _CACHE["last_result"] = res

    out = np.empty((B, E, N, N), dtype=np.float32)
    for k in range(NCORES):
        b = k // (NCORES // B)
        e0 = (k % (NCORES // B)) * EPC
        out[b, e0:e0 + EPC] = res.results[k]["out"].astype(np.float32)
    return out


# revision 6
# speedup vs baseline: 1.6025x; 1.0786x over previous
"""Trainium2 Bass/Tile kernel for the GatedNode2Edge op.

Computes, for emb (B,C,N), th12_* (E,C), th5_* (E,):
    t_k  = th12_k @ emb[b]                      (E,N)
    m_k  = max(t_k[:,i], t_k[:,j]) pairwise     (E,N,N)
    adj  = relu(2*m_1 + th5_1*I)
    gate = sigmoid(relu(2*m_2 + th5_2*I))
    out  = adj * gate                           (B,E,N,N)

Sharding: the 64 (b,e) channels are split 8-per-core across 8 NeuronCores.

Math restructuring (off-diagonal):
    relu(2*max(a,b)) = max(2*relu(a), 2*relu(b))           (relu monotone)
    sigmoid(max(x,y)) = max(sigmoid(x), sigmoid(y))        (sigmoid monotone)
    sigmoid(2*relu(t)) = max(sigmoid(2t), 1/2)             (relu fold)
so with row vectors v = 2*relu(t1), g = sigmoid(2*t2):
    out[i,j] = max(v_i, v_j) * max(g_i, g_j, 1/2)
one fused custom-DVE op per [128, N] output tile:
    out = maxx(Src0, C0) * maxx(Src1, C1)
with Src0 = v broadcast across partitions (PE outer-product), C0 = v column
slice (per-partition scalar), Src1 = sigmoid(2t2) broadcast, and C1 =
max(g_i, 1/2) column slice (the relu fold rides in the scalar).

Performance structure vs the f32 baseline:
  - All pairwise tiles are fp16: halves both the DVE stream work and the
    HBM output traffic. Host upcasts to f32.
  - The custom DVE op carries a hand-authored 2X_1PORT uop program
    (packed SRC/SRC_HI fp16 pairs -> WR0_LO/WR0_HI) and each instruction
    sets perf_max=1: 2 output elements/cycle.
  - All matmuls run in fp16 (single-pass PE) instead of fp32 (LOW_HIGH
    double-pass), shrinking the serial phase-1/2 head.
  - The true diagonal is patched with ONE strided copy_predicated per
    channel (free AP [(8 blocks, 1152), (128, 1)]).
  - Output leaves per channel as a single 2 MB DMA ([128, 8192] SBUF ->
    [8x128, 1024] DRAM 3D AP) for near-line-rate HBM writes.
"""

import sys
import types

import numpy as np

B, C, N, E = 2, 64, 1024, 32
NCORES = 8
EPC = B * E // NCORES  # 8 channels per core
P = 128
NB = N // P  # 8 row blocks
H = N // 2  # matmul moving free-dim limit is 512

PERF2X = True  # set False to fall back to 1x custom-DVE mode

_CACHE = {}


def _ensure_hook_shim():
    """Make trace=True safe even when antenv.axon_hooks is absent."""
    try:
        import antenv.axon_hooks  # noqa: F401
    except ImportError:
        mod = types.ModuleType("antenv.axon_hooks")
        mod.get_axon_ntff_profile_hook = lambda: None
        mod.set_axon_ntff_profile_hook = lambda h: None
        sys.modules["antenv.axon_hooks"] = mod


def _build_2x_uop():
    """2X_1PORT program for out = max(Src0,C0)*max(Src1,C1).

    Per cycle the engine delivers packed fp16 pairs: (SRC_0, SRC_0_HI) from
    rd0 and (SRC_1, SRC_1_HI) from rd1.  Six ALU blocks compute both
    results; the even (lo) result rides delay chain 0 to the writeback,
    the odd (hi) result arrives through the block-7 ALU bypass.
    """
    from concourse.dve_uop import (
        AluInp,
        AluOp,
        DelayInp,
        InpSel,
        OutPath,
        OutSel,
        Trigger,
        UopConfig,
    )

    u = UopConfig()
    # input lanes 1..6 feed block 0's delay chains 0..5
    u.enable_input(InpSel.SRC_0, 1)      # chain 0
    u.enable_input(InpSel.CONST_0, 2)    # chain 1
    u.enable_input(InpSel.SRC_1, 3)      # chain 2
    u.enable_input(InpSel.CONST_1, 4)    # chain 3
    u.enable_input(InpSel.SRC_0_HI, 5)   # chain 4
    u.enable_input(InpSel.SRC_1_HI, 6)   # chain 5
    dp = u.datapath_config
    # b0: m0 = max(S0, C0); latch all operands into the delay chains
    dp[0].enable_alu(AluOp.MAX, AluInp.PREV_DELAY_0, AluInp.PREV_DELAY_1)
    dp[0].pass_through_delay(1, 2, 3, 4, 5)
    # b1: m1 = max(S1, C1); chain0 <- m0
    dp[1].enable_alu(AluOp.MAX, AluInp.PREV_DELAY_2, AluInp.PREV_DELAY_3)
    dp[1].enable_delay_from_src(DelayInp.PREV_ALU_OUT, 0)
    dp[1].pass_through_delay(1, 3, 4, 5)
    # b2: r_lo = m1 * m0
    dp[2].enable_alu(AluOp.MULTIPLY, AluInp.PREV_ALU_OUT, AluInp.PREV_DELAY_0)
    dp[2].pass_through_delay(1, 3, 4, 5)
    # b3: m2 = max(S0_HI, C0); chain0 <- r_lo
    dp[3].enable_alu(AluOp.MAX, AluInp.PREV_DELAY_4, AluInp.PREV_DELAY_1)
    dp[3].enable_delay_from_src(DelayInp.PREV_ALU_OUT, 0)
    dp[3].pass_through_delay(3, 5)
    # b4: m3 = max(S1_HI, C1); chain2 <- m2; carry r_lo
    dp[4].enable_alu(AluOp.MAX, AluInp.PREV_DELAY_5, AluInp.PREV_DELAY_3)
    dp[4].enable_delay_from_src(DelayInp.PREV_ALU_OUT, 2)
    dp[4].pass_through_delay(0)
    # b5: r_hi = m3 * m2; carry r_lo
    dp[5].enable_alu(AluOp.MULTIPLY, AluInp.PREV_ALU_OUT, AluInp.PREV_DELAY_2)
    dp[5].pass_through_delay(0)
    # b6/b7: bypass r_hi down the pipe; carry r_lo
    dp[6].pass_through_alu()
    dp[6].pass_through_delay(0)
    dp[7].pass_through_alu()
    dp[7].pass_through_delay(0)
    u.enable_output(OutSel.DELAY_0, OutPath.WR0_LO)
    u.enable_output(OutSel.ALU_OUT, OutPath.WR0_HI)
    u.require_inp0 = 1
    u.require_inp1 = 1
    u.trigger = (Trigger.SRC_TENSOR_DONE, Trigger.NONE, Trigger.NONE)
    return u


def _register_gated_maxmul():
    """Register the fused out = max(in0,s0)*max(in1,s1) custom DVE op,
    with both the compiler-lowered 1x program and the hand-built 2x one."""
    import concourse.dve_ops as dve_ops
    from concourse.dve_ops import _COMPILE_CACHE, OPS, DveOp, has_src1
    from concourse.dve_spec import C0, C1, Spec, Src0, Src1, lower, maxx
    from concourse.dve_uop import DveOpSpec

    NAME = "GATED_MAXMUL2X_ANT"
    for op in OPS:
        if op.name == NAME:
            return op

    spec = Spec(
        body=maxx(Src0, C0) * maxx(Src1, C1),
        reference=lambda in0, in1, s0, s1, imm2: np.maximum(in0, s0)
        * np.maximum(in1, s1),
    )
    op = DveOp(NAME, spec, subdim=False, uops_sha={})
    OPS.append(op)
    # Rebuild the registry views that were snapshotted at import time.
    dve_ops.CUSTOM_DVE_SPECS[op.name] = op.spec
    opcode = dve_ops._CUSTOM_DVE_ROW_BASE + len(OPS) - 1
    assert opcode < 0x20
    dve_ops._SUB_OPCODE_FOR_NAME[op.name] = opcode
    # TRN2 is DVE ver v3. Pre-seed the compile cache with a spec carrying
    # the 2x program; pin the sha self-consistently.
    s = DveOpSpec(
        name=op.name,
        opcode=opcode,
        uops=lower(spec, ver="v3"),
        uops_2x=[_build_2x_uop()] if PERF2X else None,
        perf_max=1 if PERF2X else 0,
        rd1_en=has_src1(spec),
    )
    op.uops_sha["v3"] = s.sha("v3")
    _COMPILE_CACHE[(op.name, "v3")] = s
    return op


def _build_program():
    import concourse.bacc as bacc
    import concourse.mybir as mybir
    import concourse.tile as tile
    from concourse.ap import AP

    dt = mybir.dt.float32
    f16 = mybir.dt.float16
    AF = mybir.ActivationFunctionType
    ALU = mybir.AluOpType

    gated_op = _register_gated_maxmul()

    nc = bacc.Bacc("TRN2", target_bir_lowering=False, debug=False, num_devices=NCORES)

    emb16 = nc.declare_dram_parameter("emb16", [C, N], f16, isOutput=False)
    w1t16 = nc.declare_dram_parameter("w1t16", [C, EPC], f16, isOutput=False)
    w2t16 = nc.declare_dram_parameter("w2t16", [C, EPC], f16, isOutput=False)
    th5c1 = nc.declare_dram_parameter("th5c1", [EPC, 1], dt, isOutput=False)
    th5c2 = nc.declare_dram_parameter("th5c2", [EPC, 1], dt, isOutput=False)
    eyeT = nc.declare_dram_parameter("eyeT", [EPC, EPC], dt, isOutput=False)
    eye16 = nc.declare_dram_parameter("eye16", [P, P], f16, isOutput=False)
    ones16 = nc.declare_dram_parameter("ones16", [1, P], f16, isOutput=False)
    out = nc.declare_dram_parameter("out", [EPC, N, N], f16, isOutput=True)

    with tile.TileContext(nc, pool_alloc_mode="queue") as tc:
        with (
            tc.tile_pool(name="const", bufs=1) as cpool,
            tc.tile_pool(name="rows", bufs=1) as rpool,
        ):
            sb_emb16 = cpool.tile([C, N], f16)
            nc.sync.dma_start(out=sb_emb16[:], in_=emb16[:])
            sb_w1t = cpool.tile([C, EPC], f16)
            nc.sync.dma_start(out=sb_w1t[:], in_=w1t16[:])
            sb_w2t = cpool.tile([C, EPC], f16)
            nc.sync.dma_start(out=sb_w2t[:], in_=w2t16[:])
            sb_th5c1 = cpool.tile([EPC, 1], dt)
            nc.sync.dma_start(out=sb_th5c1[:], in_=th5c1[:])
            sb_th5c2 = cpool.tile([EPC, 1], dt)
            nc.sync.dma_start(out=sb_th5c2[:], in_=th5c2[:])
            sb_eyeT = cpool.tile([EPC, EPC], dt)
            nc.sync.dma_start(out=sb_eyeT[:], in_=eyeT[:])
            sb_eye16 = cpool.tile([P, P], f16)
            nc.sync.dma_start(out=sb_eye16[:], in_=eye16[:])
            sb_ones16 = cpool.tile([1, P], f16)
            nc.sync.dma_start(out=sb_ones16[:], in_=ones16[:])

            # Row-layout intermediates (channel on partition, node on free).
            # v rows in [:, :N], g rows in [:, N:] so one region feeds the
            # per-channel staging slices.
            sb_vg16 = rpool.tile([EPC, 2 * N], f16)
            sb_dtrue = rpool.tile([EPC, N], dt)  # true diagonal values, f32
            # Column layouts: [p, r*EPC + ch] = value at node r*128+p.
            sb_vcol = rpool.tile([P, NB * EPC], dt)
            sb_gcol = rpool.tile([P, NB * EPC], dt)
            sb_dcol16 = rpool.tile([P, NB * EPC], f16)
            # Staging row: all channels' v|g rows flattened onto partition 0.
            sb_flat = rpool.tile([1, EPC * 2 * N], f16)

            with (
                tc.tile_pool(name="ph1ps", bufs=1, space="PSUM") as p1ps,
                tc.tile_pool(name="ph1sb", bufs=1) as p1sb,
            ):
                ps_t1 = p1ps.tile([EPC, N], dt)
                ps_t2 = p1ps.tile([EPC, N], dt)
                for h in range(2):
                    nc.tensor.matmul(
                        ps_t1[:, h * H:(h + 1) * H],
                        lhsT=sb_w1t[:],
                        rhs=sb_emb16[:, h * H:(h + 1) * H],
                        start=True,
                        stop=True,
                    )
                    nc.tensor.matmul(
                        ps_t2[:, h * H:(h + 1) * H],
                        lhsT=sb_w2t[:],
                        rhs=sb_emb16[:, h * H:(h + 1) * H],
                        start=True,
                        stop=True,
                    )
                # v row = 2*relu(t1); g row = sigmoid(2*t2) (relu folded
                # into the per-partition scalar as max(.., 0.5)).
                nc.scalar.activation(sb_vg16[:, :N], ps_t1[:], AF.Relu, scale=2.0)
                nc.scalar.activation(sb_vg16[:, N:], ps_t2[:], AF.Sigmoid, scale=2.0)
                # True diagonal: relu(2t1+th5_1) * max(sigmoid(2t2+th5_2), 1/2)
                sb_d1 = p1sb.tile([EPC, N], dt)
                nc.scalar.activation(
                    sb_d1[:], ps_t1[:], AF.Relu, bias=sb_th5c1[:], scale=2.0
                )
                sb_d2 = p1sb.tile([EPC, N], dt)
                nc.scalar.activation(
                    sb_d2[:], ps_t2[:], AF.Sigmoid, bias=sb_th5c2[:], scale=2.0
                )
                nc.vector.scalar_tensor_tensor(
                    sb_dtrue[:], sb_d2[:], 0.5, sb_d1[:], ALU.max, ALU.mult
                )

                # Stage all channels' v|g rows onto partition 0 now; the
                # replication matmuls pick slices per channel later.
                for ch in range(EPC):
                    fo = ch * 2 * N
                    nc.sync.dma_start(
                        out=sb_flat[0:1, fo:fo + 2 * N], in_=sb_vg16[ch:ch + 1, :]
                    )

            with (
                tc.tile_pool(name="colps", bufs=2, space="PSUM") as cps,
                tc.tile_pool(name="colsb", bufs=4) as csb,
            ):
                # v/g columns straight from emb: t_col[r] = emb_blk.T @ w.
                # Emitted before the transposes so the emb-only work fills
                # the head while phase-1 rows are still in flight.
                for r in range(NB):
                    pv = cps.tile([P, EPC], dt, tag="pv")
                    nc.tensor.matmul(
                        pv[:], lhsT=sb_emb16[:, r * P:(r + 1) * P], rhs=sb_w1t[:],
                        start=True, stop=True,
                    )
                    nc.scalar.activation(
                        sb_vcol[:, r * EPC:(r + 1) * EPC], pv[:], AF.Relu, scale=2.0
                    )
                    pg = cps.tile([P, EPC], dt, tag="pg")
                    nc.tensor.matmul(
                        pg[:], lhsT=sb_emb16[:, r * P:(r + 1) * P], rhs=sb_w2t[:],
                        start=True, stop=True,
                    )
                    ug = csb.tile([P, EPC], dt, tag="ug")
                    nc.scalar.activation(ug[:], pg[:], AF.Sigmoid, scale=2.0)
                    nc.vector.tensor_scalar_max(
                        sb_gcol[:, r * EPC:(r + 1) * EPC], ug[:], 0.5
                    )
                # dcol transposes the dtrue rows (phase-1 dependent).
                for r in range(NB):
                    pt_c = cps.tile([P, EPC], dt, tag="pt_c")
                    nc.tensor.transpose(
                        pt_c[:], sb_dtrue[:, r * P:(r + 1) * P], sb_eyeT[:]
                    )
                    nc.scalar.copy(sb_dcol16[:, r * EPC:(r + 1) * EPC], pt_c[:])

            with (
                tc.tile_pool(name="jrepps", bufs=2, space="PSUM") as jps,
                tc.tile_pool(name="jrepsb", bufs=3) as jsb,
                tc.tile_pool(name="obuf", bufs=2) as opool,
            ):
                eye_ap = sb_eye16[:]
                dcol_ap = sb_dcol16[:]
                for ch in range(EPC):
                    # PE needs base partition 0 for both matmul operands;
                    # replicate this channel's staged v|g rows across
                    # partitions with K=1 matmuls.
                    fo = ch * 2 * N
                    ps_v = jps.tile([P, N], dt, tag="ps_v")
                    ps_g = jps.tile([P, N], dt, tag="ps_g")
                    for h in range(2):
                        nc.tensor.matmul(
                            ps_v[:, h * H:(h + 1) * H],
                            lhsT=sb_ones16[:],
                            rhs=sb_flat[0:1, fo + h * H:fo + (h + 1) * H],
                            start=True,
                            stop=True,
                        )
                        nc.tensor.matmul(
                            ps_g[:, h * H:(h + 1) * H],
                            lhsT=sb_ones16[:],
                            rhs=sb_flat[0:1, fo + N + h * H:fo + N + (h + 1) * H],
                            start=True,
                            stop=True,
                        )
                    sb_vj = jsb.tile([P, N], f16, tag="sb_vj")
                    nc.scalar.copy(sb_vj[:], ps_v[:])
                    sb_gj = jsb.tile([P, N], f16, tag="sb_gj")
                    nc.scalar.copy(sb_gj[:], ps_g[:])

                    o = opool.tile([P, NB * N], f16, tag="o")
                    for r in range(NB):
                        ci = r * EPC + ch
                        inst = nc.vector._custom_dve(
                            gated_op,
                            out=o[:, r * N:(r + 1) * N],
                            in0=sb_vj[:],
                            in1=sb_gj[:],
                            s0=sb_vcol[:, ci:ci + 1],
                            s1=sb_gcol[:, ci:ci + 1],
                        )
                        if PERF2X:
                            inst.ins.perf_max = 1

                    # One strided patch for the channel's 8 diagonal blocks:
                    # block r's diagonal segment lives at free offset
                    # r*N + r*P, length P.
                    o_ap = o[:]
                    patch_out = AP(
                        o_ap.tensor,
                        o_ap.offset,
                        [list(o_ap.ap[0]), [N + P, NB], [1, P]],
                    )
                    patch_mask = AP(
                        eye_ap.tensor,
                        eye_ap.offset,
                        [list(eye_ap.ap[0]), [0, NB], [1, P]],
                    ).bitcast(mybir.dt.int16)
                    patch_data = AP(
                        dcol_ap.tensor,
                        dcol_ap.offset + ch,
                        [list(dcol_ap.ap[0]), [EPC, NB], [0, P]],
                    )
                    nc.vector.copy_predicated(patch_out, patch_mask, patch_data)

                    # Whole channel leaves in one 2 MB DMA:
                    # SBUF [p, r*N + j] -> DRAM [ch, r*P + p, j].
                    src = AP(
                        o_ap.tensor,
                        o_ap.offset,
                        [list(o_ap.ap[0]), [N, NB], [1, N]],
                    )
                    out_ap = out[:]
                    dst = AP(
                        out_ap.tensor,
                        ch * N * N,
                        [[N, P], [P * N, NB], [1, N]],
                    )
                    nc.sync.dma_start(out=dst, in_=src)

    nc.compile()
    return nc


def _get_program():
    if "nc" not in _CACHE:
        _CACHE["nc"] = _build_program()
    return _CACHE["nc"]


def kernel(**inputs):
    _ensure_hook_shim()
    from concourse.bass_utils import run_bass_kernel_spmd

    emb = np.asarray(inputs["emb"], dtype=np.float32)
    th12_1 = np.asarray(inputs["th12_1"], dtype=np.float32)
    th12_2 = np.asarray(inputs["th12_2"], dtype=np.float32)
    th5_1 = np.asarray(inputs["th5_1"], dtype=np.float32)
    th5_2 = np.asarray(inputs["th5_2"], dtype=np.float32)
    eyeT = np.eye(EPC, dtype=np.float32)
    eye16 = np.eye(P, dtype=np.float16)
    ones16 = np.ones((1, P), dtype=np.float16)

    in_maps = []
    for k in range(NCORES):
        b = k // (NCORES // B)
        e0 = (k % (NCORES // B)) * EPC
        in_maps.append(
            {
                "emb16": np.ascontiguousarray(emb[b].astype(np.float16)),
                "w1t16": np.ascontiguousarray(th12_1[e0:e0 + EPC].T.astype(np.float16)),
                "w2t16": np.ascontiguousarray(th12_2[e0:e0 + EPC].T.astype(np.float16)),
                "th5c1": np.ascontiguousarray(th5_1[e0:e0 + EPC, None]),
                "th5c2": np.ascontiguousarray(th5_2[e0:e0 + EPC, None]),
                "eyeT": eyeT,
                "eye16": eye16,
                "ones16": ones16,
            }
        )

    nc = _get_program()
    res = run_bass_kernel_spmd(nc, in_maps, core_ids=list(range(NCORES)))
    _CACHE["last_result"] = res

    out = np.empty((B, E, N, N), dtype=np.float32)
    for k in range(NCORES):
        b = k // (NCORES // B)
        e0 = (k % (NCORES // B)) * EPC
        out[b, e0:e0 + EPC] = res.results[k]["out"].astype(np.float32)
    return out
